# revision 1
# baseline (speedup 1.0000x reference)
"""Bass/Trainium2 kernel for nn_BakaAttention: 8-way data-parallel over batch.

Per core (one batch element):
  q = rope(x@wq, off=1024); k = rope(concat(past_k, x@wk), off=0); v = concat(past_v, x@wv)
  out = softmax(mask(q k^T / 16)) v @ wo

Layouts on chip: qT/kT are feature-major [f, t]; scores computed transposed
[s, t] so PV consumes probs directly as the stationary operand; softmax
row-sums ride along as a 257th "ones" column of the moving v operand.
All matmuls run in float32r (full PE rate at N>=256, ~1e-4 rel err).
"""

import numpy as np

B, T, P, H, DH, DIN, DOUT = 8, 1024, 1024, 4, 256, 1024, 1152
S = P + T  # 2048 keys
THETA = 10000.0
NCORES = 8


def _host_constants():
    m = np.arange(0, DH, 2, dtype=np.float64) / DH          # 128 freqs
    inv = 1.0 / (THETA ** m)                                # [128]
    pos = np.arange(S, dtype=np.float64)                    # [2048]
    ang = np.outer(inv, pos)                                # [128, 2048]
    cos_full = np.cos(ang)
    sin_full = np.sin(ang)
    r = np.arange(128) // 2
    consts = {
        "cos_lo": cos_full[r, :].astype(np.float32),
        "cos_hi": cos_full[64 + r, :].astype(np.float32),
        "sin_lo": sin_full[r, :].astype(np.float32),
        "sin_hi": sin_full[64 + r, :].astype(np.float32),
    }
    prot = np.zeros((128, 128), np.float32)
    for mm in range(64):
        prot[2 * mm, 2 * mm + 1] = 1.0
        prot[2 * mm + 1, 2 * mm] = -1.0
    consts["prot"] = prot
    consts["ident"] = np.eye(128, dtype=np.float32)
    # masks[ci][sl, tl] = 1.0 if sl <= tl - 128*ci else 0 (keep), ci in 0..3
    sl = np.arange(128)[:, None]
    tl = np.arange(512)[None, :]
    masks = np.stack(
        [(sl <= tl - 128 * ci).astype(np.float32) for ci in range(4)], axis=1
    )  # [128, 4, 512]
    consts["masks"] = np.ascontiguousarray(masks)
    consts["ones"] = np.ones((128, 4), np.float32)
    consts["onesr"] = np.ones((1, 128), np.float32)
    return consts


def build_kernel(debug=False):
    import concourse.bass as bass
    import concourse.mybir as mybir
    from concourse import bacc
    from concourse.tile import TileContext

    f32 = mybir.dt.float32
    f32r = mybir.dt.float32r
    AF = mybir.ActivationFunctionType
    OP = mybir.AluOpType

    nc = bacc.Bacc(None, target_bir_lowering=False)

    x_d = nc.dram_tensor("x", [T, DIN], f32r, kind="ExternalInput")
    pk_d = nc.dram_tensor("past_k", [P, H, DH], f32r, kind="ExternalInput")
    pv_d = nc.dram_tensor("past_v", [P, H, DH], f32r, kind="ExternalInput")
    wq_d = nc.dram_tensor("wq", [DIN, DIN], f32r, kind="ExternalInput")
    wk_d = nc.dram_tensor("wk", [DIN, DIN], f32r, kind="ExternalInput")
    wv_d = nc.dram_tensor("wv", [DIN, DIN], f32r, kind="ExternalInput")
    wo_d = nc.dram_tensor("wo", [DIN, DOUT], f32r, kind="ExternalInput")
    cos_lo_d = nc.dram_tensor("cos_lo", [128, S], f32, kind="ExternalInput")
    cos_hi_d = nc.dram_tensor("cos_hi", [128, S], f32, kind="ExternalInput")
    sin_lo_d = nc.dram_tensor("sin_lo", [128, S], f32, kind="ExternalInput")
    sin_hi_d = nc.dram_tensor("sin_hi", [128, S], f32, kind="ExternalInput")
    prot_d = nc.dram_tensor("prot", [128, 128], f32r, kind="ExternalInput")
    ident_d = nc.dram_tensor("ident", [128, 128], f32r, kind="ExternalInput")
    masks_d = nc.dram_tensor("masks", [128, 4, 512], f32, kind="ExternalInput")
    ones_d = nc.dram_tensor("ones", [128, 4], f32r, kind="ExternalInput")
    onesr_d = nc.dram_tensor("onesr", [1, 128], f32r, kind="ExternalInput")
    out_d = nc.dram_tensor("out", [T, DOUT], f32, kind="ExternalOutput")
    vkind = dict(kind="ExternalOutput") if debug else {}
    v_r = nc.dram_tensor("v_r", [T, DIN], f32r, **vkind)
    qT_r = nc.dram_tensor("qT_r", [8, 128, T], f32r, **vkind)
    if debug:
        kT_dump = nc.dram_tensor("kT_dump", [8, 128, S], f32r, kind="ExternalOutput")
        y_dump = nc.dram_tensor("y_dump", [8, 128, DIN], f32, kind="ExternalOutput")

    from contextlib import ExitStack
    stack = ExitStack()
    with TileContext(nc) as tc, stack:
        cstp = stack.enter_context(tc.tile_pool(name="consts", bufs=1))
        prot = cstp.tile([128, 128], f32r, name="prot", tag="prot")
        ident = cstp.tile([128, 128], f32r, name="ident", tag="ident")
        masks = cstp.tile([128, 4, 512], f32, name="masks", tag="masks")
        ones_sb = cstp.tile([128, 4], f32r, name="ones_sb", tag="ones_sb")
        nc.sync.dma_start(out=ones_sb[:], in_=ones_d[:])
        onesr_sb = cstp.tile([1, 128], f32r, name="onesr_sb", tag="onesr_sb")
        nc.sync.dma_start(out=onesr_sb[:], in_=onesr_d[:])
        nc.sync.dma_start(out=prot[:], in_=prot_d[:])
        nc.sync.dma_start(out=ident[:], in_=ident_d[:])
        nc.sync.dma_start(out=masks[:], in_=masks_d[:])

        resid = stack.enter_context(tc.tile_pool(name="resid", bufs=1))
        kT = [resid.tile([128, S], f32r, name=f"kT{i}", tag=f"kT{i}") for i in range(8)]

        # ---------------- Phase 1+2: xT, projections, rope ----------------
        with tc.tile_pool(name="tables", bufs=1) as tabp, \
             tc.tile_pool(name="p2xT", bufs=1) as xtp, \
             tc.tile_pool(name="p2", bufs=2) as p2p, \
             tc.tile_pool(name="p2st", bufs=3) as stp, \
             tc.tile_pool(name="p2ps", bufs=4, space="PSUM") as ps2, \
             tc.tile_pool(name="p2rot", bufs=2, space="PSUM") as rotps, \
             tc.tile_pool(name="p2kp", bufs=1) as kpp:
            cos_t = [tabp.tile([128, T], f32, name="clo", tag="clo"),
                     tabp.tile([128, T], f32, name="chi", tag="chi")]
            sin_t = [tabp.tile([128, T], f32, name="slo", tag="slo"),
                     tabp.tile([128, T], f32, name="shi", tag="shi")]

            def load_tables(p0):
                nc.sync.dma_start(out=cos_t[0][:], in_=cos_lo_d[:, p0:p0 + T])
                nc.sync.dma_start(out=cos_t[1][:], in_=cos_hi_d[:, p0:p0 + T])
                nc.sync.dma_start(out=sin_t[0][:], in_=sin_lo_d[:, p0:p0 + T])
                nc.sync.dma_start(out=sin_t[1][:], in_=sin_hi_d[:, p0:p0 + T])

            load_tables(P)  # positions 1024..2047 for q and new-k

            xT = [xtp.tile([128, T], f32r, name=f"xT{i}", tag=f"xT{i}") for i in range(8)]
            for tt in range(8):
                xt = p2p.tile([128, DIN], f32r, name="xload", tag="xload")
                nc.sync.dma_start(out=xt[:], in_=x_d[128 * tt:128 * (tt + 1), :])
                for kt in range(8):
                    tp = ps2.tile([128, 128], f32, name="tps", tag="tps", bufs=2)
                    nc.tensor.matmul(tp[:], xt[:, 128 * kt:128 * (kt + 1)], ident[:],
                                     start=True, stop=True)
                    nc.scalar.copy(xT[kt][:, 128 * tt:128 * (tt + 1)], tp[:])

            def rope_combine(dst_ap, raw_sb, rot_ps, ft, off, n):
                # dst = raw * cos + rot * sin ; table rows by f-tile parity
                ctab = cos_t[ft % 2][:, off:off + n]
                stab = sin_t[ft % 2][:, off:off + n]
                t1 = p2p.tile([128, 512], f32, name="ropet1", tag="ropet1")
                nc.gpsimd.tensor_tensor(t1[:, :n], raw_sb, ctab, op=OP.mult)
                t2 = p2p.tile([128, 512], f32, name="ropet2", tag="ropet2")
                nc.vector.tensor_tensor(t2[:, :n], rot_ps, stab, op=OP.mult)
                nc.vector.tensor_tensor(dst_ap, t1[:, :n], t2[:, :n], op=OP.add)

            # q and new-k projections (transposed layout) + rope
            for w_d, dst in ((wq_d, None), (wk_d, kT)):
                for ftg in range(4):          # pairs of f-tiles
                    psl = [ps2.tile([128, 512], f32, name=f"pj{i}", tag=f"pj{i}", bufs=1) for i in range(4)]
                    for kt in range(8):
                        wt = stp.tile([128, 256], f32r, name="wload", tag="wload")
                        nc.sync.dma_start(
                            out=wt[:],
                            in_=w_d[128 * kt:128 * (kt + 1), 256 * ftg:256 * (ftg + 1)])
                        for f2 in range(2):
                            for th in range(2):
                                nc.tensor.matmul(
                                    psl[2 * f2 + th][:],
                                    wt[:, 128 * f2:128 * (f2 + 1)].bitcast(f32r),
                                    xT[kt][:, 512 * th:512 * (th + 1)].bitcast(f32r),
                                    start=(kt == 0), stop=(kt == 7))
                    for f2 in range(2):
                        ft = 2 * ftg + f2
                        raw = p2p.tile([128, 1024], f32r, name="rawsb", tag="rawsb")
                        for th in range(2):
                            nc.scalar.copy(raw[:, 512 * th:512 * (th + 1)],
                                           psl[2 * f2 + th][:])
                        if dst is None:
                            qstage = p2p.tile([128, 1024], f32r, name="qstage",
                                              tag="qstage")
                        for th in range(2):
                            rp = rotps.tile([128, 512], f32, name="rotps", tag="rotps")
                            nc.tensor.matmul(rp[:], prot[:].bitcast(f32r),
                                             raw[:, 512 * th:512 * (th + 1)].bitcast(f32r),
                                             start=True, stop=True)
                            if dst is None:
                                dst_ap = qstage[:, 512 * th:512 * (th + 1)]
                            else:
                                dst_ap = dst[ft][:, P + 512 * th:P + 512 * (th + 1)]
                            rope_combine(dst_ap, raw[:, 512 * th:512 * (th + 1)],
                                         rp[:], ft, 512 * th, 512)
                        if dst is None:
                            nc.sync.dma_start(out=qT_r[ftg * 2 + f2], in_=qstage[:])

            # v projection, natural layout [s, f] -> DRAM
            for stg in range(4):
                psl = [ps2.tile([128, 512], f32, name=f"pv{i}", tag=f"pj{i}", bufs=1) for i in range(4)]
                for kt in range(8):
                    wt = stp.tile([128, 1024], f32r, name="wvload", tag="wvload")
                    nc.sync.dma_start(out=wt[:], in_=wv_d[128 * kt:128 * (kt + 1), :])
                    for s2 in range(2):
                        st = 2 * stg + s2
                        for fh in range(2):
                            nc.tensor.matmul(
                                psl[2 * s2 + fh][:],
                                xT[kt][:, 128 * st:128 * (st + 1)].bitcast(f32r),
                                wt[:, 512 * fh:512 * (fh + 1)].bitcast(f32r),
                                start=(kt == 0), stop=(kt == 7))
                for s2 in range(2):
                    st = 2 * stg + s2
                    vsb = p2p.tile([128, 1024], f32r, name="vsb", tag="vsb")
                    for fh in range(2):
                        nc.scalar.copy(vsb[:, 512 * fh:512 * (fh + 1)],
                                       psl[2 * s2 + fh][:])
                    nc.sync.dma_start(out=v_r[128 * st:128 * (st + 1), :], in_=vsb[:])

            # past_k: transpose + rope into kT[:, 0:1024]
            load_tables(0)  # positions 0..1023
            for h in range(4):
                kp = [kpp.tile([128, P], f32r, name=f"kp{i}", tag=f"kp{i}") for i in range(2)]
                for st in range(8):
                    pkt = stp.tile([128, DH], f32r, name="pkload", tag="pkload")
                    nc.sync.dma_start(out=pkt[:],
                                      in_=pk_d[128 * st:128 * (st + 1), h, :])
                    for f2 in range(2):
                        tp = ps2.tile([128, 128], f32, name="tps", tag="tps", bufs=2)
                        nc.tensor.matmul(tp[:], pkt[:, 128 * f2:128 * (f2 + 1)],
                                         ident[:], start=True, stop=True)
                        nc.scalar.copy(kp[f2][:, 128 * st:128 * (st + 1)], tp[:])
                for f2 in range(2):
                    ft = 2 * h + f2
                    for sh in range(2):
                        rp = rotps.tile([128, 512], f32, name="rotps", tag="rotps")
                        nc.tensor.matmul(rp[:], prot[:].bitcast(f32r),
                                         kp[f2][:, 512 * sh:512 * (sh + 1)].bitcast(f32r),
                                         start=True, stop=True)
                        rope_combine(kT[ft][:, 512 * sh:512 * (sh + 1)],
                                     kp[f2][:, 512 * sh:512 * (sh + 1)],
                                     rp[:], ft, 512 * sh, 512)

        if debug:
            for i in range(8):
                nc.sync.dma_start(out=kT_dump[i], in_=kT[i][:])

        # ---------------- Phase 3: attention ----------------
        ysbp = stack.enter_context(tc.tile_pool(name="ysb", bufs=1))
        yT = [ysbp.tile([128, T], f32r, name=f"yT{i}", tag=f"yT{i}")
              for i in range(8)]
        with tc.tile_pool(name="vaug", bufs=1) as vap, \
             tc.tile_pool(name="qth", bufs=2) as qtp, \
             tc.tile_pool(name="probs", bufs=5) as prp, \
             tc.tile_pool(name="p3sm", bufs=4) as smp, \
             tc.tile_pool(name="p3sc", bufs=3, space="PSUM") as scps, \
             tc.tile_pool(name="p3y", bufs=1, space="PSUM") as yps:
            for h in range(4):
                qh = [qtp.tile([128, T], f32r, name=f"qh{fk}", tag=f"qh{fk}")
                      for fk in range(2)]
                for fk in range(2):
                    nc.sync.dma_start(out=qh[fk][:], in_=qT_r[2 * h + fk])
                va = [vap.tile([128, 260], f32r, name=f"va{j}", tag=f"va{j}")
                      for j in range(16)]
                for j in range(16):
                    if j < 8:
                        src = pv_d[128 * j:128 * (j + 1), h, :]
                    else:
                        src = v_r[128 * (j - 8):128 * (j - 7),
                                  DH * h:DH * (h + 1)]
                    nc.sync.dma_start(out=va[j][:, 0:DH], in_=src)
                for TH in range(2):
                    jmax = 12 + 4 * TH
                    ytp_ps = [yps.tile([128, 512], f32, name=f"ytp{i}",
                                       tag=f"ytp{i}", bufs=1) for i in range(2)]
                    sm_ps = yps.tile([1, 512], f32, name="smps", tag="smps",
                                     bufs=1)
                    for j in range(jmax):
                        sc = scps.tile([128, 512], f32, name="sc", tag="sc")
                        for fk in range(2):
                            nc.tensor.matmul(
                                sc[:],
                                kT[2 * h + fk][:, 128 * j:128 * (j + 1)].bitcast(f32r),
                                qh[fk][:, 512 * TH:512 * (TH + 1)].bitcast(f32r),
                                start=(fk == 0), stop=(fk == 1))
                        pj = prp.tile([128, 512], f32r, name="pj", tag="pj")
                        nc.scalar.activation(pj[:], sc[:], AF.Exp, scale=float(DH ** -0.5))
                        ci = j - (8 + 4 * TH)
                        if ci >= 0:
                            nc.gpsimd.tensor_tensor(pj[:], pj[:], masks[:, ci, :],
                                                    op=OP.mult)
                        for fb in range(2):
                            nc.tensor.matmul(
                                ytp_ps[fb][:],
                                va[j][:, 128 * fb:128 * (fb + 1)],
                                pj[:],
                                start=(j == 0), stop=(j == jmax - 1))
                        nc.tensor.matmul(
                            sm_ps[:], ones_sb[:, 0:1], pj[:],
                            start=(j == 0), stop=(j == jmax - 1))
                    rc = smp.tile([1, 512], f32r, name="rc", tag="rc")
                    with nc.allow_low_precision(reason="f32r bits == f32"):
                        nc.vector.reciprocal(rc[:], sm_ps[:])
                    bc_ps = scps.tile([128, 512], f32, name="bcps", tag="bcps",
                                      bufs=1)
                    nc.tensor.matmul(bc_ps[:], onesr_sb[:], rc[:],
                                     start=True, stop=True)
                    bc_sb = smp.tile([128, 512], f32, name="bcsb", tag="bcsb")
                    nc.scalar.copy(bc_sb[:], bc_ps[:])
                    for fb in range(2):
                        nc.vector.tensor_tensor(
                            yT[2 * h + fb][:, 512 * TH:512 * (TH + 1)],
                            ytp_ps[fb][:],
                            bc_sb[:],
                            op=OP.mult)

        # ---------------- Phase 4: o-projection ----------------
        with tc.tile_pool(name="p4wo", bufs=1) as wop, \
             tc.tile_pool(name="p4o", bufs=2) as osp, \
             tc.tile_pool(name="p4ps", bufs=4, space="PSUM") as ps4:
            wo_sb = [wop.tile([128, DOUT], f32r, name=f"wo{i}", tag=f"wo{i}")
                     for i in range(8)]
            for kt in range(8):
                nc.sync.dma_start(out=wo_sb[kt][:],
                                  in_=wo_d[128 * kt:128 * (kt + 1), :])
            for tt in range(8):
                ot = osp.tile([128, DOUT], f32, name="osb", tag="osb")
                for ds in range(3):
                    op_ps = ps4.tile([128, 384], f32, name="ops", tag="ops", bufs=3)
                    for fk in range(8):
                        nc.tensor.matmul(
                            op_ps[:],
                            yT[fk][:, 128 * tt:128 * (tt + 1)],
                            wo_sb[fk][:, 384 * ds:384 * (ds + 1)],
                            start=(fk == 0), stop=(fk == 7))
                    nc.scalar.copy(ot[:, 384 * ds:384 * (ds + 1)], op_ps[:])
                nc.sync.dma_start(out=out_d[128 * tt:128 * (tt + 1), :], in_=ot[:])

    nc.finalize()
    return nc


_NC_CACHE = {}


def run(x, past_k, past_v, wq, wk, wv, wo, debug=False, trace=False):
    from concourse.bass_utils import run_bass_kernel_spmd

    key = (debug,)
    if key not in _NC_CACHE:
        _NC_CACHE[key] = build_kernel(debug=debug)
    nc = _NC_CACHE[key]
    consts = _host_constants()
    in_maps = []
    for b in range(NCORES):
        m = {
            "x": np.ascontiguousarray(x[b]),
            "past_k": np.ascontiguousarray(past_k[b]),
            "past_v": np.ascontiguousarray(past_v[b]),
            "wq": wq, "wk": wk, "wv": wv, "wo": wo,
            "cos_lo": consts["cos_lo"], "cos_hi": consts["cos_hi"],
            "sin_lo": consts["sin_lo"], "sin_hi": consts["sin_hi"],
            "prot": consts["prot"], "ident": consts["ident"],
            "masks": consts["masks"], "ones": consts["ones"], "onesr": consts["onesr"],
        }
        in_maps.append(m)
    res = run_bass_kernel_spmd(nc, in_maps, list(range(NCORES)), trace=trace)
    out = np.stack([res.results[b]["out"] for b in range(NCORES)], axis=0)
    return out, res


def kernel(x, past_k, past_v, wq, wk, wv, wo):
    out, _ = run(x, past_k, past_v, wq, wk, wv, wo)
    return out



# revision 2
# speedup vs baseline: 1.8110x; 1.8110x over previous
"""Bass/Trainium2 kernel for nn_BakaAttention: 8-way data-parallel over batch.

Per core (one batch element):
  q = rope(x@wq, off=1024); k = concat(rope_host(past_k), rope(x@wk));
  v = concat(past_v, x@wv); out = softmax(mask(q k^T / 16)) v @ wo

All matmuls run in bf16 (1 cycle/row on the PE vs 2 for f32r's fp32-HIGH
mode). x is pre-transposed and past_k pre-roped+transposed on the host, so
the device does no transposes. Everything stays SBUF-resident (no DRAM
spills). Scores are computed transposed [keys, queries] so PV consumes the
probs directly as the moving operand; softmax row-sums accumulate via a
ones-column matmul and the normalization uses the fast custom-DVE
reciprocal off the critical path.
"""

import numpy as np

B, T, P, H, DH, DIN, DOUT = 8, 1024, 1024, 4, 256, 1024, 1152
S = P + T  # 2048 keys
THETA = 10000.0
NCORES = 8


def _host_prep(x, past_k, past_v, wq, wk, wv, wo):
    """Per-batch input prep: bf16 casts, transposes, past_k rope."""
    import ml_dtypes

    bf16 = ml_dtypes.bfloat16

    m = np.arange(0, DH, 2, dtype=np.float64) / DH          # 128 freqs
    inv = 1.0 / (THETA ** m)                                # [128]

    # past_k rope at offset 0, interleaved pairs
    pos = np.arange(P, dtype=np.float64)
    ang = np.outer(pos, inv)                                # [P, 128]
    c = np.cos(ang)[:, None, :]                             # [P, 1, 128]
    s = np.sin(ang)[:, None, :]
    pk = past_k.astype(np.float64)                          # [B, P, H, DH]
    x1, x2 = pk[..., 0::2], pk[..., 1::2]
    o1 = x1 * c - x2 * s
    o2 = x2 * c + x1 * s
    pk_rot = np.stack([o1, o2], axis=-1).reshape(B, P, H, DH)
    # kT layout: [B, 8 ftile, 128, P]; ftile ft=2h+f2 covers head-local
    # features 128*f2 + p
    pkT = np.ascontiguousarray(
        pk_rot.reshape(B, P, 8, 128).transpose(0, 2, 3, 1)
    ).astype(bf16)

    # device rope tables for positions P..P+T-1 (used by both q and new-k)
    posq = np.arange(P, P + T, dtype=np.float64)
    angq = np.outer(inv, posq)                              # [128 m, T]
    cosq, sinq = np.cos(angq), np.sin(angq)
    r = np.arange(128) // 2
    tabs = {
        "cos_lo": cosq[r, :].astype(bf16),
        "cos_hi": cosq[64 + r, :].astype(bf16),
        "sin_lo": sinq[r, :].astype(bf16),
        "sin_hi": sinq[64 + r, :].astype(bf16),
    }

    # pair-rotation matrix: rot = prot.T @ raw -> rot[2m] = -raw[2m+1],
    # rot[2m+1] = raw[2m]
    prot = np.zeros((128, 128), np.float32)
    for mm in range(64):
        prot[2 * mm, 2 * mm + 1] = 1.0
        prot[2 * mm + 1, 2 * mm] = -1.0

    # masks[p, ci, tl] = 1 if key p within diag block ci is visible to
    # local query tl
    sl = np.arange(128)[:, None]
    tl = np.arange(512)[None, :]
    masks = np.stack(
        [(sl <= tl - 128 * ci).astype(np.float32) for ci in range(4)], axis=1
    )

    common = {
        "wqp": np.ascontiguousarray(
            wq.reshape(8, 128, 4, 256).transpose(2, 0, 1, 3)).astype(bf16),
        "wkp": np.ascontiguousarray(
            wk.reshape(8, 128, 4, 256).transpose(2, 0, 1, 3)).astype(bf16),
        "wv": wv.reshape(8, 128, DIN).astype(bf16),
        "wo": wo.reshape(8, 128, DOUT).astype(bf16),
        "prot": prot.astype(bf16),
        "masks": np.ascontiguousarray(masks).astype(bf16),
        "ones": np.ones((128, 1), bf16),
        "onesr": np.ones((1, 128), bf16),
        **tabs,
    }

    per_core = []
    for b in range(NCORES):
        per_core.append({
            "xT": np.ascontiguousarray(x[b].T).astype(bf16),
            "pkT": pkT[b],
            "pvf": np.ascontiguousarray(
                past_v[b].reshape(P, DIN).reshape(8, 128, DIN)).astype(bf16),
            **common,
        })
    return per_core


def build_kernel():
    import concourse.bass as bass  # noqa: F401
    import concourse.mybir as mybir
    from concourse import bacc
    from concourse.tile import TileContext

    f32 = mybir.dt.float32
    bf = mybir.dt.bfloat16
    AF = mybir.ActivationFunctionType
    OP = mybir.AluOpType

    nc = bacc.Bacc(None, target_bir_lowering=False)

    xT_d = nc.dram_tensor("xT", [DIN, T], bf, kind="ExternalInput")
    wqp_d = nc.dram_tensor("wqp", [4, 8, 128, 256], bf, kind="ExternalInput")
    wkp_d = nc.dram_tensor("wkp", [4, 8, 128, 256], bf, kind="ExternalInput")
    wv_d = nc.dram_tensor("wv", [8, 128, DIN], bf, kind="ExternalInput")
    wo_d = nc.dram_tensor("wo", [8, 128, DOUT], bf, kind="ExternalInput")
    pkT_d = nc.dram_tensor("pkT", [8, 128, P], bf, kind="ExternalInput")
    pvf_d = nc.dram_tensor("pvf", [8, 128, DIN], bf, kind="ExternalInput")
    tab_d = {n: nc.dram_tensor(n, [128, T], bf, kind="ExternalInput")
             for n in ("cos_lo", "cos_hi", "sin_lo", "sin_hi")}
    prot_d = nc.dram_tensor("prot", [128, 128], bf, kind="ExternalInput")
    masks_d = nc.dram_tensor("masks", [128, 4, 512], bf, kind="ExternalInput")
    ones_d = nc.dram_tensor("ones", [128, 1], bf, kind="ExternalInput")
    onesr_d = nc.dram_tensor("onesr", [1, 128], bf, kind="ExternalInput")
    out_d = nc.dram_tensor("out", [T, DOUT], f32, kind="ExternalOutput")

    from contextlib import ExitStack
    stack = ExitStack()
    with TileContext(nc) as tc, stack:
        cst = stack.enter_context(tc.tile_pool(name="consts", bufs=1))
        dat = stack.enter_context(tc.tile_pool(name="data", bufs=1))
        wtp = stack.enter_context(tc.tile_pool(name="wt", bufs=1))
        rawp = stack.enter_context(tc.tile_pool(name="raw", bufs=4))
        pjp = stack.enter_context(tc.tile_pool(name="pj", bufs=6))
        smlp = stack.enter_context(tc.tile_pool(name="sml", bufs=2))
        otp = stack.enter_context(tc.tile_pool(name="ot", bufs=3))
        quad = stack.enter_context(tc.tile_pool(name="quad", bufs=1, space="PSUM"))
        mxp = stack.enter_context(tc.tile_pool(name="mx", bufs=3, space="PSUM"))
        ytpp = stack.enter_context(tc.tile_pool(name="ytp", bufs=1, space="PSUM"))
        smbc = stack.enter_context(tc.tile_pool(name="smbc", bufs=1, space="PSUM"))

        # ---- persistent SBUF tiles + input DMAs ----
        xT = [dat.tile([128, T], bf, name=f"xT{i}", tag=f"xT{i}") for i in range(8)]
        for kt in range(8):
            nc.sync.dma_start(out=xT[kt][:], in_=xT_d[128 * kt:128 * (kt + 1), :])

        cos_t = [cst.tile([128, T], bf, name=n, tag=n) for n in ("clo", "chi")]
        sin_t = [cst.tile([128, T], bf, name=n, tag=n) for n in ("slo", "shi")]
        for t_sb, n in zip(cos_t + sin_t,
                           ("cos_lo", "cos_hi", "sin_lo", "sin_hi")):
            nc.sync.dma_start(out=t_sb[:], in_=tab_d[n][:])
        prot = cst.tile([128, 128], bf, name="prot", tag="prot")
        nc.sync.dma_start(out=prot[:], in_=prot_d[:])
        ones_sb = cst.tile([128, 1], bf, name="ones", tag="ones")
        nc.sync.dma_start(out=ones_sb[:], in_=ones_d[:])
        onesr_sb = cst.tile([1, 128], bf, name="onesr", tag="onesr")
        nc.sync.dma_start(out=onesr_sb[:], in_=onesr_d[:])
        masks = cst.tile([128, 4, 512], bf, name="masks", tag="masks")
        nc.sync.dma_start(out=masks[:], in_=masks_d[:])

        kT = [dat.tile([128, S], bf, name=f"kT{i}", tag=f"kT{i}") for i in range(8)]
        for ft in range(8):
            nc.sync.dma_start(out=kT[ft][:, 0:P], in_=pkT_d[ft])
        pv_sb = [dat.tile([128, DIN], bf, name=f"pv{i}", tag=f"pv{i}")
                 for i in range(8)]
        for st in range(8):
            nc.sync.dma_start(out=pv_sb[st][:], in_=pvf_d[st])

        qT = [dat.tile([128, T], bf, name=f"qT{i}", tag=f"qT{i}") for i in range(8)]
        v_sb = [dat.tile([128, DIN], bf, name=f"v{i}", tag=f"v{i}")
                for i in range(8)]
        yT = [dat.tile([128, T], bf, name=f"yT{i}", tag=f"yT{i}") for i in range(8)]

        # ---- building blocks ----
        def rope_combine(dst_ap, raw_sb, rot_ps, f2, t0):
            # dst = raw * cos + rot * sin, tables sliced at local t0
            ctab = cos_t[f2][:, t0:t0 + 512]
            stab = sin_t[f2][:, t0:t0 + 512]
            t1 = rawp.tile([128, 512], bf, name="ropet1", tag="ropet1")
            nc.gpsimd.tensor_tensor(t1[:], raw_sb, ctab, op=OP.mult)
            t2 = rawp.tile([128, 512], bf, name="ropet2", tag="ropet2")
            nc.vector.tensor_tensor(t2[:], rot_ps, stab, op=OP.mult)
            nc.vector.tensor_tensor(dst_ap, t1[:], t2[:], op=OP.add)

        def proj_qk(h, w_d, dst, dst_off):
            # dst[2h+f2][:, dst_off + t] = rope(x @ w)[128f2+p, t]
            wts = [wtp.tile([128, 256], bf, name="wqk", tag=f"wqk{kt}", bufs=2)
                   for kt in range(8)]
            for kt in range(8):
                nc.sync.dma_start(out=wts[kt][:], in_=w_d[h, kt])
            for f2 in range(2):
                psl = [quad.tile([128, 512], f32, name="psl", tag=tg)
                       for tg in ("qa", "qb")]
                for kt in range(8):
                    for th in range(2):
                        nc.tensor.matmul(
                            psl[th][:],
                            wts[kt][:, 128 * f2:128 * (f2 + 1)],
                            xT[kt][:, 512 * th:512 * (th + 1)],
                            start=(kt == 0), stop=(kt == 7))
                for th in range(2):
                    raw = rawp.tile([128, 512], bf, name="raw", tag="raw")
                    nc.scalar.copy(raw[:], psl[th][:])
                    rp = mxp.tile([128, 512], f32, name="rot", tag="mx")
                    nc.tensor.matmul(rp[:], prot[:], raw[:],
                                     start=True, stop=True)
                    rope_combine(
                        dst[2 * h + f2][:, dst_off + 512 * th:
                                        dst_off + 512 * (th + 1)],
                        raw[:], rp[:], f2, 512 * th)

        def proj_v():
            wvs = [wtp.tile([128, DIN], bf, name="wv", tag=f"wv{kt}")
                   for kt in range(8)]
            for kt in range(8):
                nc.sync.dma_start(out=wvs[kt][:], in_=wv_d[kt])
            for st in range(8):
                for fh in range(2):
                    pv_ps = quad.tile([128, 512], f32, name="pvps",
                                      tag=("qa", "qb")[fh])
                    for kt in range(8):
                        nc.tensor.matmul(
                            pv_ps[:],
                            xT[kt][:, 128 * st:128 * (st + 1)],
                            wvs[kt][:, 512 * fh:512 * (fh + 1)],
                            start=(kt == 0), stop=(kt == 7))
                    nc.vector.tensor_copy(
                        v_sb[st][:, 512 * fh:512 * (fh + 1)], pv_ps[:])

        def vsrc(j, h, fb):
            src = pv_sb[j] if j < 8 else v_sb[j - 8]
            c0 = 256 * h + 128 * fb
            return src[:, c0:c0 + 128]

        def attention(h):
            for TH in range(2):
                jmax = 12 + 4 * TH
                ytp = [ytpp.tile([128, 512], f32, name=f"ytp{i}", tag=f"y{i}")
                       for i in range(2)]
                sm = smbc.tile([128, 512], f32, name="sm", tag="sb")
                for j in range(jmax):
                    sc = mxp.tile([128, 512], f32, name="sc", tag="mx")
                    for fk in range(2):
                        nc.tensor.matmul(
                            sc[:],
                            kT[2 * h + fk][:, 128 * j:128 * (j + 1)],
                            qT[2 * h + fk][:, 512 * TH:512 * (TH + 1)],
                            start=(fk == 0), stop=(fk == 1))
                    pj = pjp.tile([128, 512], bf, name="pj", tag="pj")
                    nc.scalar.activation(pj[:], sc[:], AF.Exp,
                                         scale=float(DH ** -0.5))
                    ci = j - (8 + 4 * TH)
                    if ci >= 0:
                        nc.vector.tensor_tensor(pj[:], pj[:], masks[:, ci, :],
                                                op=OP.mult)
                    for fb in range(2):
                        nc.tensor.matmul(ytp[fb][:], vsrc(j, h, fb), pj[:],
                                         start=(j == 0), stop=(j == jmax - 1))
                    nc.tensor.matmul(sm[0:1, :], ones_sb[:], pj[:],
                                     start=(j == 0), stop=(j == jmax - 1))
                rc = smlp.tile([1, 512], f32, name="rc", tag="rc")
                nc.vector.reciprocal_approx_fast(out=rc[:], in_=sm[0:1, :])
                rcb = smlp.tile([1, 512], bf, name="rcb", tag="rcb")
                nc.scalar.copy(rcb[:], rc[:])
                bc_ps = smbc.tile([128, 512], f32, name="bc", tag="sb")
                nc.tensor.matmul(bc_ps[:], onesr_sb[:], rcb[:],
                                 start=True, stop=True)
                bc_sb = smlp.tile([128, 512], bf, name="bcsb", tag="bcsb")
                nc.scalar.copy(bc_sb[:], bc_ps[:])
                for fb in range(2):
                    nc.vector.tensor_tensor(
                        yT[2 * h + fb][:, 512 * TH:512 * (TH + 1)],
                        ytp[fb][:], bc_sb[:], op=OP.mult)

        # ---- program ----
        proj_qk(0, wqp_d, qT, 0)
        proj_qk(0, wkp_d, kT, P)
        proj_v()
        attention(0)
        wo_sb = [dat.tile([128, DOUT], bf, name=f"wo{i}", tag=f"wo{i}")
                 for i in range(8)]
        for kt in range(8):
            nc.sync.dma_start(out=wo_sb[kt][:], in_=wo_d[kt])
        for h in range(1, 4):
            proj_qk(h, wqp_d, qT, 0)
            proj_qk(h, wkp_d, kT, P)
            attention(h)

        for tt in range(8):
            for ds in range(3):
                op_ps = quad.tile([128, 512], f32, name="ops",
                                  tag=("qa", "qb")[ds % 2])
                for fk in range(8):
                    nc.tensor.matmul(
                        op_ps[:, 0:384],
                        yT[fk][:, 128 * tt:128 * (tt + 1)],
                        wo_sb[fk][:, 384 * ds:384 * (ds + 1)],
                        start=(fk == 0), stop=(fk == 7))
                ot = otp.tile([128, 384], f32, name="ot", tag="ot")
                nc.vector.tensor_copy(ot[:], op_ps[:, 0:384])
                nc.sync.dma_start(
                    out=out_d[128 * tt:128 * (tt + 1), 384 * ds:384 * (ds + 1)],
                    in_=ot[:])

    nc.finalize()
    return nc


_NC_CACHE = {}


def run(x, past_k, past_v, wq, wk, wv, wo, debug=False, trace=False):
    from concourse.bass_utils import run_bass_kernel_spmd

    if "nc" not in _NC_CACHE:
        _NC_CACHE["nc"] = build_kernel()
    nc = _NC_CACHE["nc"]
    in_maps = _host_prep(x, past_k, past_v, wq, wk, wv, wo)
    res = run_bass_kernel_spmd(nc, in_maps, list(range(NCORES)), trace=trace)
    out = np.stack([res.results[b]["out"] for b in range(NCORES)], axis=0)
    return out.astype(np.float32), res


def kernel(x, past_k, past_v, wq, wk, wv, wo):
    out, _ = run(x, past_k, past_v, wq, wk, wv, wo)
    return out


# revision 6
# speedup vs baseline: 1.9168x; 1.0584x over previous
"""Bass/Trainium2 kernel for nn_BakaAttention: 8-way data-parallel over batch.

Per core (one batch element):
  q = rope(x@wq, off=1024); k = concat(rope_host(past_k), rope(x@wk));
  v = concat(past_v, x@wv); out = softmax(mask(q k^T / 16)) v @ wo

All matmuls run in bf16 (1 cycle/row on the PE vs 2 for f32r's fp32-HIGH
mode). x is pre-transposed and past_k pre-roped+transposed on the host, so
the device does no transposes. Everything stays SBUF-resident (no DRAM
spills). Scores are computed transposed [keys, queries] so PV consumes the
probs directly as the moving operand; softmax row-sums accumulate via a
ones-column matmul and the normalization uses the fast custom-DVE
reciprocal off the critical path.
"""

import numpy as np

B, T, P, H, DH, DIN, DOUT = 8, 1024, 1024, 4, 256, 1024, 1152
S = P + T  # 2048 keys
THETA = 10000.0
NCORES = 8


def _host_prep(x, past_k, past_v, wq, wk, wv, wo):
    """Per-batch input prep: bf16 casts, transposes, past_k rope."""
    import ml_dtypes

    bf16 = ml_dtypes.bfloat16

    m = np.arange(0, DH, 2, dtype=np.float64) / DH          # 128 freqs
    inv = 1.0 / (THETA ** m)                                # [128]

    # past_k rope at offset 0, interleaved pairs
    pos = np.arange(P, dtype=np.float64)
    ang = np.outer(pos, inv)                                # [P, 128]
    c = np.cos(ang)[:, None, :]                             # [P, 1, 128]
    s = np.sin(ang)[:, None, :]
    pk = past_k.astype(np.float64)                          # [B, P, H, DH]
    x1, x2 = pk[..., 0::2], pk[..., 1::2]
    o1 = x1 * c - x2 * s
    o2 = x2 * c + x1 * s
    pk_rot = np.stack([o1, o2], axis=-1).reshape(B, P, H, DH)
    # kT layout: [B, 8 ftile, 128, P]; ftile ft=2h+f2 covers head-local
    # features 128*f2 + p
    pkT = np.ascontiguousarray(
        pk_rot.reshape(B, P, 8, 128).transpose(0, 2, 3, 1)
    ).astype(bf16)

    # device rope tables for positions P..P+T-1 (used by both q and new-k)
    posq = np.arange(P, P + T, dtype=np.float64)
    angq = np.outer(inv, posq)                              # [128 m, T]
    cosq, sinq = np.cos(angq), np.sin(angq)
    r = np.arange(128) // 2
    tabs = {
        "cos_lo": cosq[r, :].astype(bf16),
        "cos_hi": cosq[64 + r, :].astype(bf16),
        "sin_lo": sinq[r, :].astype(bf16),
        "sin_hi": sinq[64 + r, :].astype(bf16),
    }

    # pair-rotation matrix: rot = prot.T @ raw -> rot[2m] = -raw[2m+1],
    # rot[2m+1] = raw[2m]
    prot = np.zeros((128, 128), np.float32)
    for mm in range(64):
        prot[2 * mm, 2 * mm + 1] = 1.0
        prot[2 * mm + 1, 2 * mm] = -1.0

    # masks[p, ci, tl] = 1 if key p within diag block ci is visible to
    # local query tl
    sl = np.arange(128)[:, None]
    tl = np.arange(512)[None, :]
    masks = np.stack(
        [(sl <= tl - 128 * ci).astype(np.float32) for ci in range(4)], axis=1
    )

    common = {
        "wqp": np.ascontiguousarray(
            wq.reshape(8, 128, 4, 256).transpose(2, 0, 1, 3)).astype(bf16),
        "wkp": np.ascontiguousarray(
            wk.reshape(8, 128, 4, 256).transpose(2, 0, 1, 3)).astype(bf16),
        "wv": wv.reshape(8, 128, DIN).astype(bf16),
        "wo": wo.reshape(8, 128, DOUT).astype(bf16),
        "prot": prot.astype(bf16),
        "masks": np.ascontiguousarray(masks).astype(bf16),
        "ones": np.ones((128, 1), bf16),
        "onesr": np.ones((1, 128), bf16),
        **tabs,
    }

    per_core = []
    for b in range(NCORES):
        per_core.append({
            "xT": np.ascontiguousarray(x[b].T).astype(bf16),
            "pkT": pkT[b],
            "pvf": np.ascontiguousarray(
                past_v[b].reshape(P, DIN).reshape(8, 128, DIN)).astype(bf16),
            **common,
        })
    return per_core


def build_kernel():
    import concourse.bass as bass  # noqa: F401
    import concourse.mybir as mybir
    from concourse import bacc
    from concourse.tile import TileContext

    f32 = mybir.dt.float32
    bf = mybir.dt.bfloat16
    AF = mybir.ActivationFunctionType
    OP = mybir.AluOpType

    nc = bacc.Bacc(None, target_bir_lowering=False)

    xT_d = nc.dram_tensor("xT", [DIN, T], bf, kind="ExternalInput")
    wqp_d = nc.dram_tensor("wqp", [4, 8, 128, 256], bf, kind="ExternalInput")
    wkp_d = nc.dram_tensor("wkp", [4, 8, 128, 256], bf, kind="ExternalInput")
    wv_d = nc.dram_tensor("wv", [8, 128, DIN], bf, kind="ExternalInput")
    wo_d = nc.dram_tensor("wo", [8, 128, DOUT], bf, kind="ExternalInput")
    pkT_d = nc.dram_tensor("pkT", [8, 128, P], bf, kind="ExternalInput")
    pvf_d = nc.dram_tensor("pvf", [8, 128, DIN], bf, kind="ExternalInput")
    tab_d = {n: nc.dram_tensor(n, [128, T], bf, kind="ExternalInput")
             for n in ("cos_lo", "cos_hi", "sin_lo", "sin_hi")}
    prot_d = nc.dram_tensor("prot", [128, 128], bf, kind="ExternalInput")
    masks_d = nc.dram_tensor("masks", [128, 4, 512], bf, kind="ExternalInput")
    ones_d = nc.dram_tensor("ones", [128, 1], bf, kind="ExternalInput")
    onesr_d = nc.dram_tensor("onesr", [1, 128], bf, kind="ExternalInput")
    out_d = nc.dram_tensor("out", [T, DOUT], f32, kind="ExternalOutput")

    from contextlib import ExitStack
    stack = ExitStack()
    with TileContext(nc) as tc, stack:
        cst = stack.enter_context(tc.tile_pool(name="consts", bufs=1))
        dat = stack.enter_context(tc.tile_pool(name="data", bufs=1))
        wtp = stack.enter_context(tc.tile_pool(name="wt", bufs=1))
        rawp = stack.enter_context(tc.tile_pool(name="raw", bufs=4))
        pjp = stack.enter_context(tc.tile_pool(name="pj", bufs=6))
        smlp = stack.enter_context(tc.tile_pool(name="sml", bufs=2))
        otp = stack.enter_context(tc.tile_pool(name="ot", bufs=3))
        quad = stack.enter_context(tc.tile_pool(name="quad", bufs=1, space="PSUM"))
        mxp = stack.enter_context(tc.tile_pool(name="mx", bufs=3, space="PSUM"))
        ytpp = stack.enter_context(tc.tile_pool(name="ytp", bufs=1, space="PSUM"))
        smbc = stack.enter_context(tc.tile_pool(name="smbc", bufs=1, space="PSUM"))

        # ---- persistent SBUF tiles + startup DMAs ----
        # Emission order = SP issue order; the first projection matmuls need
        # wq(h0)[kt] + xT[kt], so those lead, interleaved, with the rope
        # constants woven in. Everything else follows in order of first use.
        xT = [dat.tile([128, T], bf, name=f"xT{i}", tag=f"xT{i}") for i in range(8)]
        cos_t = [cst.tile([128, T], bf, name=n, tag=n) for n in ("clo", "chi")]
        sin_t = [cst.tile([128, T], bf, name=n, tag=n) for n in ("slo", "shi")]
        prot = cst.tile([128, 128], bf, name="prot", tag="prot")
        ones_sb = cst.tile([128, 1], bf, name="ones", tag="ones")
        onesr_sb = cst.tile([1, 128], bf, name="onesr", tag="onesr")
        masks = cst.tile([128, 4, 512], bf, name="masks", tag="masks")
        kT = [dat.tile([128, S], bf, name=f"kT{i}", tag=f"kT{i}") for i in range(8)]
        pv_sb = [dat.tile([128, DIN], bf, name=f"pv{i}", tag=f"pv{i}")
                 for i in range(8)]
        qT = [dat.tile([128, T], bf, name=f"qT{i}", tag=f"qT{i}") for i in range(8)]

        wts_q0 = [wtp.tile([128, 256], bf, name="wqk", tag=f"wqk{kt}", bufs=2)
                  for kt in range(8)]
        for kt in range(8):
            nc.sync.dma_start(out=wts_q0[kt][:], in_=wqp_d[0, kt])
            nc.sync.dma_start(out=xT[kt][:], in_=xT_d[128 * kt:128 * (kt + 1), :])
            if kt == 1:
                nc.sync.dma_start(out=prot[:], in_=prot_d[:])
            elif kt == 3:
                nc.sync.dma_start(out=cos_t[0][:], in_=tab_d["cos_lo"][:])
                nc.sync.dma_start(out=sin_t[0][:], in_=tab_d["sin_lo"][:])
            elif kt == 5:
                nc.sync.dma_start(out=cos_t[1][:], in_=tab_d["cos_hi"][:])
                nc.sync.dma_start(out=sin_t[1][:], in_=tab_d["sin_hi"][:])
        nc.sync.dma_start(out=ones_sb[:], in_=ones_d[:])
        nc.sync.dma_start(out=onesr_sb[:], in_=onesr_d[:])
        for ft in range(8):
            nc.sync.dma_start(out=kT[ft][:, 0:P], in_=pkT_d[ft])
        for st in range(8):
            nc.sync.dma_start(out=pv_sb[st][:], in_=pvf_d[st])
        nc.sync.dma_start(out=masks[:], in_=masks_d[:])
        v_sb = [dat.tile([128, DIN], bf, name=f"v{i}", tag=f"v{i}")
                for i in range(8)]
        yT = [dat.tile([128, T], bf, name=f"yT{i}", tag=f"yT{i}") for i in range(8)]

        # ---- building blocks ----
        def rope_combine(dst_ap, raw_sb, rot_ps, f2, t0):
            # dst = raw * cos + rot * sin, tables sliced at local t0
            ctab = cos_t[f2][:, t0:t0 + 512]
            stab = sin_t[f2][:, t0:t0 + 512]
            t1 = rawp.tile([128, 512], bf, name="ropet1", tag="ropet1")
            nc.gpsimd.tensor_tensor(t1[:], raw_sb, ctab, op=OP.mult)
            t2 = rawp.tile([128, 512], bf, name="ropet2", tag="ropet2")
            nc.vector.tensor_tensor(t2[:], rot_ps, stab, op=OP.mult)
            nc.vector.tensor_tensor(dst_ap, t1[:], t2[:], op=OP.add)

        def proj_qk(h, w_d, dst, dst_off, wts=None):
            # dst[2h+f2][:, dst_off + t] = rope(x @ w)[128f2+p, t]
            if wts is None:
                wts = [wtp.tile([128, 256], bf, name="wqk", tag=f"wqk{kt}",
                                bufs=2) for kt in range(8)]
                for kt in range(8):
                    nc.sync.dma_start(out=wts[kt][:], in_=w_d[h, kt])
            for f2 in range(2):
                psl = [quad.tile([128, 512], f32, name="psl", tag=tg)
                       for tg in ("qa", "qb")]
                for kt in range(8):
                    for th in range(2):
                        nc.tensor.matmul(
                            psl[th][:],
                            wts[kt][:, 128 * f2:128 * (f2 + 1)],
                            xT[kt][:, 512 * th:512 * (th + 1)],
                            start=(kt == 0), stop=(kt == 7))
                for th in range(2):
                    raw = rawp.tile([128, 512], bf, name="raw", tag="raw")
                    nc.scalar.copy(raw[:], psl[th][:])
                    rp = mxp.tile([128, 512], f32, name="rot", tag="mx")
                    nc.tensor.matmul(rp[:], prot[:], raw[:],
                                     start=True, stop=True)
                    rope_combine(
                        dst[2 * h + f2][:, dst_off + 512 * th:
                                        dst_off + 512 * (th + 1)],
                        raw[:], rp[:], f2, 512 * th)

        def proj_v():
            wvs = [wtp.tile([128, DIN], bf, name="wv", tag=f"wv{kt}")
                   for kt in range(8)]
            for kt in range(8):
                nc.sync.dma_start(out=wvs[kt][:], in_=wv_d[kt])
            for st in range(8):
                for fh in range(2):
                    pv_ps = quad.tile([128, 512], f32, name="pvps",
                                      tag=("qa", "qb")[fh])
                    for kt in range(8):
                        nc.tensor.matmul(
                            pv_ps[:],
                            xT[kt][:, 128 * st:128 * (st + 1)],
                            wvs[kt][:, 512 * fh:512 * (fh + 1)],
                            start=(kt == 0), stop=(kt == 7))
                    nc.vector.tensor_copy(
                        v_sb[st][:, 512 * fh:512 * (fh + 1)], pv_ps[:])

        def vsrc(j, h, fb):
            src = pv_sb[j] if j < 8 else v_sb[j - 8]
            c0 = 256 * h + 128 * fb
            return src[:, c0:c0 + 128]

        def attention(h):
            for TH in range(2):
                jmax = 12 + 4 * TH
                ytp = [ytpp.tile([128, 512], f32, name=f"ytp{i}", tag=f"y{i}")
                       for i in range(2)]
                sm = smbc.tile([128, 512], f32, name="sm", tag="sb")
                for j in range(jmax):
                    # diagonal block ci only serves local queries >= 128*ci
                    ci = j - (8 + 4 * TH)
                    q0 = 128 * ci if ci > 0 else 0
                    sc = mxp.tile([128, 512], f32, name="sc", tag="mx")
                    for fk in range(2):
                        nc.tensor.matmul(
                            sc[:, q0:512],
                            kT[2 * h + fk][:, 128 * j:128 * (j + 1)],
                            qT[2 * h + fk][:, 512 * TH + q0:512 * (TH + 1)],
                            start=(fk == 0), stop=(fk == 1))
                    pj = pjp.tile([128, 512], bf, name="pj", tag="pj")
                    nc.scalar.activation(pj[:, q0:512], sc[:, q0:512], AF.Exp,
                                         scale=float(DH ** -0.5))
                    if ci >= 0:
                        nc.vector.tensor_tensor(
                            pj[:, q0:q0 + 128], pj[:, q0:q0 + 128],
                            masks[:, ci, q0:q0 + 128], op=OP.mult)
                    for fb in range(2):
                        nc.tensor.matmul(ytp[fb][:, q0:512],
                                         vsrc(j, h, fb), pj[:, q0:512],
                                         start=(j == 0), stop=(j == jmax - 1))
                    nc.tensor.matmul(sm[0:1, q0:512], ones_sb[:],
                                     pj[:, q0:512],
                                     start=(j == 0), stop=(j == jmax - 1))
                rc = smlp.tile([1, 512], f32, name="rc", tag="rc")
                nc.vector.reciprocal_approx_fast(out=rc[:], in_=sm[0:1, :])
                rcb = smlp.tile([1, 512], bf, name="rcb", tag="rcb")
                nc.scalar.copy(rcb[:], rc[:])
                bc_ps = smbc.tile([128, 512], f32, name="bc", tag="sb")
                nc.tensor.matmul(bc_ps[:], onesr_sb[:], rcb[:],
                                 start=True, stop=True)
                bc_sb = smlp.tile([128, 512], bf, name="bcsb", tag="bcsb")
                nc.scalar.copy(bc_sb[:], bc_ps[:])
                for fb in range(2):
                    nc.vector.tensor_tensor(
                        yT[2 * h + fb][:, 512 * TH:512 * (TH + 1)],
                        ytp[fb][:], bc_sb[:], op=OP.mult)

        # ---- program ----
        proj_qk(0, wqp_d, qT, 0, wts=wts_q0)
        proj_qk(0, wkp_d, kT, P)
        proj_v()
        attention(0)
        wo_sb = [dat.tile([128, DOUT], bf, name=f"wo{i}", tag=f"wo{i}")
                 for i in range(8)]
        for h in range(1, 4):
            proj_qk(h, wqp_d, qT, 0)
            proj_qk(h, wkp_d, kT, P)
            if h == 2:
                for kt in range(8):
                    nc.sync.dma_start(out=wo_sb[kt][:], in_=wo_d[kt])
            attention(h)

        for tt in range(8):
            for ds in range(3):
                op_ps = quad.tile([128, 512], f32, name="ops",
                                  tag=("qa", "qb")[ds % 2])
                for fk in range(8):
                    nc.tensor.matmul(
                        op_ps[:, 0:384],
                        yT[fk][:, 128 * tt:128 * (tt + 1)],
                        wo_sb[fk][:, 384 * ds:384 * (ds + 1)],
                        start=(fk == 0), stop=(fk == 7))
                ot = otp.tile([128, 384], f32, name="ot", tag="ot")
                nc.vector.tensor_copy(ot[:], op_ps[:, 0:384])
                nc.sync.dma_start(
                    out=out_d[128 * tt:128 * (tt + 1), 384 * ds:384 * (ds + 1)],
                    in_=ot[:])

    nc.finalize()
    return nc


_NC_CACHE = {}


def run(x, past_k, past_v, wq, wk, wv, wo, debug=False, trace=False):
    from concourse.bass_utils import run_bass_kernel_spmd

    if "nc" not in _NC_CACHE:
        _NC_CACHE["nc"] = build_kernel()
    nc = _NC_CACHE["nc"]
    in_maps = _host_prep(x, past_k, past_v, wq, wk, wv, wo)
    res = run_bass_kernel_spmd(nc, in_maps, list(range(NCORES)), trace=trace)
    out = np.stack([res.results[b]["out"] for b in range(NCORES)], axis=0)
    return out.astype(np.float32), res


def kernel(x, past_k, past_v, wq, wk, wv, wo):
    out, _ = run(x, past_k, past_v, wq, wk, wv, wo)
    return out


# revision 13
# speedup vs baseline: 1.9324x; 1.0081x over previous
"""Bass/Trainium2 kernel for nn_BakaAttention: 8-way data-parallel over batch.

Per core (one batch element):
  q = rope(x@wq, off=1024); k = concat(rope_host(past_k), rope(x@wk));
  v = concat(past_v, x@wv); out = softmax(mask(q k^T / 16)) v @ wo

All matmuls run in bf16 (1 cycle/row on the PE vs 2 for f32r's fp32-HIGH
mode). x is pre-transposed and past_k pre-roped+transposed on the host, so
the device does no transposes. Everything stays SBUF-resident (no DRAM
spills). Scores are computed transposed [keys, queries] so PV consumes the
probs directly as the moving operand; softmax row-sums accumulate via a
ones-column matmul and the normalization uses the fast custom-DVE
reciprocal off the critical path.
"""

import numpy as np

B, T, P, H, DH, DIN, DOUT = 8, 1024, 1024, 4, 256, 1024, 1152
S = P + T  # 2048 keys
THETA = 10000.0
NCORES = 8


def _host_prep(x, past_k, past_v, wq, wk, wv, wo):
    """Per-batch input prep: bf16 casts, transposes, past_k rope."""
    import ml_dtypes

    bf16 = ml_dtypes.bfloat16

    m = np.arange(0, DH, 2, dtype=np.float64) / DH          # 128 freqs
    inv = 1.0 / (THETA ** m)                                # [128]

    # past_k rope at offset 0, interleaved pairs
    pos = np.arange(P, dtype=np.float64)
    ang = np.outer(pos, inv)                                # [P, 128]
    c = np.cos(ang)[:, None, :]                             # [P, 1, 128]
    s = np.sin(ang)[:, None, :]
    pk = past_k.astype(np.float64)                          # [B, P, H, DH]
    x1, x2 = pk[..., 0::2], pk[..., 1::2]
    o1 = x1 * c - x2 * s
    o2 = x2 * c + x1 * s
    pk_rot = np.stack([o1, o2], axis=-1).reshape(B, P, H, DH)
    # kT layout: [B, 8 ftile, 128, P]; ftile ft=2h+f2 covers head-local
    # features 128*f2 + p
    pkT = np.ascontiguousarray(
        pk_rot.reshape(B, P, 8, 128).transpose(0, 2, 3, 1)
    ).astype(bf16)

    # device rope tables for positions P..P+T-1 (used by both q and new-k)
    posq = np.arange(P, P + T, dtype=np.float64)
    angq = np.outer(inv, posq)                              # [128 m, T]
    cosq, sinq = np.cos(angq), np.sin(angq)
    r = np.arange(128) // 2
    tabs = np.ascontiguousarray(np.stack(
        [cosq[r, :], cosq[64 + r, :], sinq[r, :], sinq[64 + r, :]], axis=1
    )).astype(bf16)  # [128, 4, T]

    # pair-rotation matrix: rot = prot.T @ raw -> rot[2m] = -raw[2m+1],
    # rot[2m+1] = raw[2m]
    prot = np.zeros((128, 128), np.float32)
    for mm in range(64):
        prot[2 * mm, 2 * mm + 1] = 1.0
        prot[2 * mm + 1, 2 * mm] = -1.0

    # masks[p, ci, tl] = 1 if key p within diag block ci is visible to
    # local query tl
    sl = np.arange(128)[:, None]
    tl = np.arange(512)[None, :]
    masks = np.stack(
        [(sl <= tl - 128 * ci).astype(np.float32) for ci in range(4)], axis=1
    )

    common = {
        "wqp": np.ascontiguousarray(
            wq.reshape(8, 128, 4, 256).transpose(2, 0, 1, 3)).astype(bf16),
        "wkp": np.ascontiguousarray(
            wk.reshape(8, 128, 4, 256).transpose(2, 0, 1, 3)).astype(bf16),
        "wv": wv.reshape(8, 128, DIN).astype(bf16),
        "wo": wo.reshape(8, 128, DOUT).astype(bf16),
        "prot": prot.astype(bf16),
        "masks": np.ascontiguousarray(masks).astype(bf16),
        "ones": np.ones((128, 1), bf16),
        "onesr": np.ones((1, 128), bf16),
        "tabs": tabs,
    }

    per_core = []
    for b in range(NCORES):
        per_core.append({
            "xT": np.ascontiguousarray(x[b].T).astype(bf16),
            "pkT": pkT[b],
            "pvf": np.ascontiguousarray(
                past_v[b].reshape(P, DIN).reshape(8, 128, DIN)).astype(bf16),
            **common,
        })
    return per_core


def build_kernel():
    import concourse.bass as bass  # noqa: F401
    import concourse.mybir as mybir
    from concourse import bacc
    from concourse.tile import TileContext

    f32 = mybir.dt.float32
    bf = mybir.dt.bfloat16
    AF = mybir.ActivationFunctionType
    OP = mybir.AluOpType

    nc = bacc.Bacc(None, target_bir_lowering=False)

    xT_d = nc.dram_tensor("xT", [DIN, T], bf, kind="ExternalInput")
    wqp_d = nc.dram_tensor("wqp", [4, 8, 128, 256], bf, kind="ExternalInput")
    wkp_d = nc.dram_tensor("wkp", [4, 8, 128, 256], bf, kind="ExternalInput")
    wv_d = nc.dram_tensor("wv", [8, 128, DIN], bf, kind="ExternalInput")
    wo_d = nc.dram_tensor("wo", [8, 128, DOUT], bf, kind="ExternalInput")
    pkT_d = nc.dram_tensor("pkT", [8, 128, P], bf, kind="ExternalInput")
    pvf_d = nc.dram_tensor("pvf", [8, 128, DIN], bf, kind="ExternalInput")
    tabs_d = nc.dram_tensor("tabs", [128, 4, T], bf, kind="ExternalInput")
    prot_d = nc.dram_tensor("prot", [128, 128], bf, kind="ExternalInput")
    masks_d = nc.dram_tensor("masks", [128, 4, 512], bf, kind="ExternalInput")
    ones_d = nc.dram_tensor("ones", [128, 1], bf, kind="ExternalInput")
    onesr_d = nc.dram_tensor("onesr", [1, 128], bf, kind="ExternalInput")
    out_d = nc.dram_tensor("out", [T, DOUT], f32, kind="ExternalOutput")

    from contextlib import ExitStack
    stack = ExitStack()
    with TileContext(nc) as tc, stack:
        cst = stack.enter_context(tc.tile_pool(name="consts", bufs=1))
        dat = stack.enter_context(tc.tile_pool(name="data", bufs=1))
        wtp = stack.enter_context(tc.tile_pool(name="wt", bufs=1))
        rawp = stack.enter_context(tc.tile_pool(name="raw", bufs=4))
        pjp = stack.enter_context(tc.tile_pool(name="pj", bufs=6))
        smlp = stack.enter_context(tc.tile_pool(name="sml", bufs=2))
        otp = stack.enter_context(tc.tile_pool(name="ot", bufs=3))
        quad = stack.enter_context(tc.tile_pool(name="quad", bufs=1, space="PSUM"))
        mxp = stack.enter_context(tc.tile_pool(name="mx", bufs=3, space="PSUM"))
        ytpp = stack.enter_context(tc.tile_pool(name="ytp", bufs=1, space="PSUM"))
        smbc = stack.enter_context(tc.tile_pool(name="smbc", bufs=1, space="PSUM"))

        # ---- persistent SBUF tiles + startup DMAs ----
        # Emission order = SP issue order; the first projection matmuls need
        # wq(h0)[kt] + xT[kt], so those lead, interleaved, with the rope
        # constants woven in. Everything else follows in order of first use.
        xT = [dat.tile([128, T], bf, name=f"xT{i}", tag=f"xT{i}") for i in range(8)]
        tabs = cst.tile([128, 4, T], bf, name="tabs", tag="tabs")
        prot = cst.tile([128, 128], bf, name="prot", tag="prot")
        ones_sb = cst.tile([128, 1], bf, name="ones", tag="ones")
        onesr_sb = cst.tile([1, 128], bf, name="onesr", tag="onesr")
        masks = cst.tile([128, 4, 512], bf, name="masks", tag="masks")
        kT = [dat.tile([128, S], bf, name=f"kT{i}", tag=f"kT{i}") for i in range(8)]
        pv_sb = [dat.tile([128, DIN], bf, name=f"pv{i}", tag=f"pv{i}")
                 for i in range(8)]
        qT = [dat.tile([128, T], bf, name=f"qT{i}", tag=f"qT{i}") for i in range(8)]

        # SP queue: q-weights + xT interleaved (first matmuls' inputs),
        # rope constants woven in early
        wts_q0 = [wtp.tile([128, 256], bf, name="wqk", tag=f"wqk{kt}", bufs=2)
                  for kt in range(8)]
        for kt in range(8):
            nc.sync.dma_start(out=wts_q0[kt][:], in_=wqp_d[0, kt])
            nc.sync.dma_start(out=xT[kt][:], in_=xT_d[128 * kt:128 * (kt + 1), :])
            if kt == 1:
                nc.sync.dma_start(out=prot[:], in_=prot_d[:])
            elif kt == 3:
                nc.sync.dma_start(out=tabs[:], in_=tabs_d[:])
        # idle-at-start engine queues absorb the remaining input DMA issues
        for ft in range(8):
            nc.gpsimd.dma_start(out=kT[ft][:, 0:P], in_=pkT_d[ft])
        for st in range(8):
            nc.scalar.dma_start(out=pv_sb[st][:], in_=pvf_d[st])
        nc.scalar.dma_start(out=ones_sb[:], in_=ones_d[:])
        nc.scalar.dma_start(out=onesr_sb[:], in_=onesr_d[:])
        nc.scalar.dma_start(out=masks[:], in_=masks_d[:])
        v_sb = [dat.tile([128, DIN], bf, name=f"v{i}", tag=f"v{i}")
                for i in range(8)]
        yT = [dat.tile([128, T], bf, name=f"yT{i}", tag=f"yT{i}") for i in range(8)]

        # ---- building blocks ----
        def rope_combine(dst_ap, raw_sb, rot_ps, f2, t0):
            # dst = raw * cos + rot * sin, tables sliced at local t0
            ctab = tabs[:, f2, t0:t0 + 512]
            stab = tabs[:, 2 + f2, t0:t0 + 512]
            t1 = rawp.tile([128, 512], bf, name="ropet1", tag="ropet1")
            nc.gpsimd.tensor_tensor(t1[:], raw_sb, ctab, op=OP.mult)
            t2 = rawp.tile([128, 512], bf, name="ropet2", tag="ropet2")
            nc.vector.tensor_tensor(t2[:], rot_ps, stab, op=OP.mult)
            nc.vector.tensor_tensor(dst_ap, t1[:], t2[:], op=OP.add)

        def proj_qk(h, w_d, dst, dst_off, wts=None):
            # dst[2h+f2][:, dst_off + t] = rope(x @ w)[128f2+p, t]
            if wts is None:
                wts = [wtp.tile([128, 256], bf, name="wqk", tag=f"wqk{kt}",
                                bufs=2) for kt in range(8)]
                for kt in range(8):
                    nc.sync.dma_start(out=wts[kt][:], in_=w_d[h, kt])
            for f2 in range(2):
                psl = [quad.tile([128, 512], f32, name="psl", tag=tg)
                       for tg in ("qa", "qb")]
                for kt in range(8):
                    for th in range(2):
                        nc.tensor.matmul(
                            psl[th][:],
                            wts[kt][:, 128 * f2:128 * (f2 + 1)],
                            xT[kt][:, 512 * th:512 * (th + 1)],
                            start=(kt == 0), stop=(kt == 7))
                for th in range(2):
                    raw = rawp.tile([128, 512], bf, name="raw", tag="raw")
                    nc.scalar.copy(raw[:], psl[th][:])
                    rp = mxp.tile([128, 512], f32, name="rot", tag="mx")
                    nc.tensor.matmul(rp[:], prot[:], raw[:],
                                     start=True, stop=True)
                    rope_combine(
                        dst[2 * h + f2][:, dst_off + 512 * th:
                                        dst_off + 512 * (th + 1)],
                        raw[:], rp[:], f2, 512 * th)

        def proj_v():
            wvs = [wtp.tile([128, DIN], bf, name="wv", tag=f"wv{kt}")
                   for kt in range(8)]
            for kt in range(8):
                nc.sync.dma_start(out=wvs[kt][:], in_=wv_d[kt])
            for st in range(8):
                for fh in range(2):
                    pv_ps = quad.tile([128, 512], f32, name="pvps",
                                      tag=("qa", "qb")[fh])
                    for kt in range(8):
                        nc.tensor.matmul(
                            pv_ps[:],
                            xT[kt][:, 128 * st:128 * (st + 1)],
                            wvs[kt][:, 512 * fh:512 * (fh + 1)],
                            start=(kt == 0), stop=(kt == 7))
                    nc.vector.tensor_copy(
                        v_sb[st][:, 512 * fh:512 * (fh + 1)], pv_ps[:])

        def vsrc(j, h, fb):
            src = pv_sb[j] if j < 8 else v_sb[j - 8]
            c0 = 256 * h + 128 * fb
            return src[:, c0:c0 + 128]

        def attention(h):
            for TH in range(2):
                jmax = 12 + 4 * TH
                ytp = [ytpp.tile([128, 512], f32, name=f"ytp{i}", tag=f"y{i}")
                       for i in range(2)]
                sm = smbc.tile([128, 512], f32, name="sm", tag="sb")
                for j in range(jmax):
                    # diagonal block ci only serves local queries >= 128*ci
                    ci = j - (8 + 4 * TH)
                    q0 = 128 * ci if ci > 0 else 0
                    sc = mxp.tile([128, 512], f32, name="sc", tag="mx")
                    for fk in range(2):
                        nc.tensor.matmul(
                            sc[:, q0:512],
                            kT[2 * h + fk][:, 128 * j:128 * (j + 1)],
                            qT[2 * h + fk][:, 512 * TH + q0:512 * (TH + 1)],
                            start=(fk == 0), stop=(fk == 1))
                    pj = pjp.tile([128, 512], bf, name="pj", tag="pj")
                    nc.scalar.activation(pj[:, q0:512], sc[:, q0:512], AF.Exp,
                                         scale=float(DH ** -0.5))
                    if ci >= 0:
                        nc.vector.tensor_tensor(
                            pj[:, q0:q0 + 128], pj[:, q0:q0 + 128],
                            masks[:, ci, q0:q0 + 128], op=OP.mult)
                    for fb in range(2):
                        nc.tensor.matmul(ytp[fb][:, q0:512],
                                         vsrc(j, h, fb), pj[:, q0:512],
                                         start=(j == 0), stop=(j == jmax - 1))
                    nc.tensor.matmul(sm[0:1, q0:512], ones_sb[:],
                                     pj[:, q0:512],
                                     start=(j == 0), stop=(j == jmax - 1))
                rc = smlp.tile([1, 512], f32, name="rc", tag="rc")
                nc.vector.reciprocal_approx_fast(out=rc[:], in_=sm[0:1, :])
                rcb = smlp.tile([1, 512], bf, name="rcb", tag="rcb")
                nc.scalar.copy(rcb[:], rc[:])
                bc_ps = smbc.tile([128, 512], f32, name="bc", tag="sb")
                nc.tensor.matmul(bc_ps[:], onesr_sb[:], rcb[:],
                                 start=True, stop=True)
                bc_sb = smlp.tile([128, 512], bf, name="bcsb", tag="bcsb")
                nc.scalar.copy(bc_sb[:], bc_ps[:])
                for fb in range(2):
                    nc.vector.tensor_tensor(
                        yT[2 * h + fb][:, 512 * TH:512 * (TH + 1)],
                        ytp[fb][:], bc_sb[:], op=OP.mult)

        # ---- program ----
        proj_qk(0, wqp_d, qT, 0, wts=wts_q0)
        proj_qk(0, wkp_d, kT, P)
        proj_v()
        attention(0)
        wo_sb = [dat.tile([128, DOUT], bf, name=f"wo{i}", tag=f"wo{i}")
                 for i in range(8)]
        for h in range(1, 4):
            proj_qk(h, wqp_d, qT, 0)
            proj_qk(h, wkp_d, kT, P)
            if h == 2:
                for kt in range(8):
                    nc.sync.dma_start(out=wo_sb[kt][:], in_=wo_d[kt])
            attention(h)

        for tt in range(8):
            for ds in range(3):
                op_ps = quad.tile([128, 512], f32, name="ops",
                                  tag=("qa", "qb")[ds % 2])
                for fk in range(8):
                    nc.tensor.matmul(
                        op_ps[:, 0:384],
                        yT[fk][:, 128 * tt:128 * (tt + 1)],
                        wo_sb[fk][:, 384 * ds:384 * (ds + 1)],
                        start=(fk == 0), stop=(fk == 7))
                ot = otp.tile([128, 384], f32, name="ot", tag="ot")
                nc.vector.tensor_copy(ot[:], op_ps[:, 0:384])
                nc.sync.dma_start(
                    out=out_d[128 * tt:128 * (tt + 1), 384 * ds:384 * (ds + 1)],
                    in_=ot[:])

    nc.finalize()
    return nc


_NC_CACHE = {}


def run(x, past_k, past_v, wq, wk, wv, wo, debug=False, trace=False):
    from concourse.bass_utils import run_bass_kernel_spmd

    if "nc" not in _NC_CACHE:
        _NC_CACHE["nc"] = build_kernel()
    nc = _NC_CACHE["nc"]
    in_maps = _host_prep(x, past_k, past_v, wq, wk, wv, wo)
    res = run_bass_kernel_spmd(nc, in_maps, list(range(NCORES)), trace=trace)
    out = np.stack([res.results[b]["out"] for b in range(NCORES)], axis=0)
    return out.astype(np.float32), res


def kernel(x, past_k, past_v, wq, wk, wv, wo):
    out, _ = run(x, past_k, past_v, wq, wk, wv, wo)
    return out
